# revision 6
# baseline (speedup 1.0000x reference)
"""Trainium2 Bass kernel for single-token-decode causal self-attention
(nn_CausalSelfAttention): qkv proj + RoPE + KV-cache update + SDPA + out proj.

Sharding: heads across 8 cores (2 heads/core, all 16 batches). Each core
computes its heads' attention and a partial output projection; the host sums
the 8 partials (the "all-reduce") and splices the new k/v row into the
full-size cache passthrough outputs.

Self-contained: hardcodes B=16, T=1, C=2048, H=16, D=128, MAX_SEQ=4096 and
expects mask = prefix of length pos+1 with input_pos = pos (the decode-step
contract of the reference). Falls back to a pure-numpy path if the inputs
don't match that contract.
"""

import math
import sys

sys.path.insert(0, "/opt/trn_rl_repo")

import numpy as np
import ml_dtypes

B = 16
C = 2048
NH = 16
D = 128
MAX_SEQ = 4096
N_CORES = 8
HEADS_PER_CORE = NH // N_CORES  # 2
PAIRS = B * HEADS_PER_CORE      # 32 (b,h)-pairs per core; pair index p = h*16 + b
SCALE = 1.0 / math.sqrt(D)

BF16 = ml_dtypes.bfloat16

_cache = {}


def _build_program(S):
    """Builds the SPMD Bass/Tile program for attention over S cache rows
    (S % 512 == 0) plus the freshly-computed token. Returns compiled nc.

    Everything runs in a transposed layout (seq / head-dim on partitions,
    (b,h)-pairs on the free axis) so every matmul output starts at PSUM
    partition 0:
      scoresT[s, t, p] = K_p[t*128+s, :] . q_p     (K-tile stationary)
      eT = exp(scoresT); denom[1, p] = ones . eT   (ones stationary)
      yT[d, p] = sum_t V_p_tile.T @ eT[:, t, p] + vP.T @ diag(e_new)
      y_pair = yT.T * rinv;  y_part = y_pair.T.T @ Wp  (per-head slices)
    """
    import concourse.bass as bass
    import concourse.tile as tile
    from concourse import bacc, mybir
    from concourse.masks import make_identity

    NT = S // 128          # number of 128-row seq tiles
    KT = C // 128          # qkv contraction tiles (16)
    F3 = 3 * HEADS_PER_CORE * D  # 768 qkv output cols per core
    HB = HEADS_PER_CORE * B      # == PAIRS

    fp32 = mybir.dt.float32
    bf16 = mybir.dt.bfloat16
    Exp = mybir.ActivationFunctionType.Exp

    nc = bacc.Bacc("TRN2", target_bir_lowering=False, debug=False,
                   enable_asserts=False, num_devices=N_CORES)

    # ---- DRAM I/O (per-core contents supplied by host) ----
    xt_d = nc.dram_tensor("xt", [128, KT, B], bf16, kind="ExternalInput").ap()
    wc_d = nc.dram_tensor("wc", [128, KT, F3], bf16, kind="ExternalInput").ap()
    wp_d = nc.dram_tensor("wp", [128, HEADS_PER_CORE, C], bf16,
                          kind="ExternalInput").ap()
    kt_d = nc.dram_tensor("kt", [B, HEADS_PER_CORE, 128, S], bf16,
                          kind="ExternalInput").ap()
    vt_d = nc.dram_tensor("vt", [B, HEADS_PER_CORE, 128, NT, 128], bf16,
                          kind="ExternalInput").ap()
    cs_d = nc.dram_tensor("cs", [B, 4, 128], fp32, kind="ExternalInput").ap()

    yp_d = nc.dram_tensor("yp", [B, C], fp32, kind="ExternalOutput").ap()
    knew_d = nc.dram_tensor("knew", [B, 256], fp32, kind="ExternalOutput").ap()
    vnew_d = nc.dram_tensor("vnew", [B, 256], fp32, kind="ExternalOutput").ap()

    with tile.TileContext(nc) as tc:
        with (
            tc.tile_pool(name="consts", bufs=1) as consts,
            tc.tile_pool(name="kpool", bufs=3) as kpool,
            tc.tile_pool(name="vpool", bufs=3) as vpool,
            tc.tile_pool(name="sb", bufs=1) as sb,
            tc.tile_pool(name="psum_big", bufs=1, space="PSUM") as psum_big,
            tc.tile_pool(name="psum_yt", bufs=1, space="PSUM") as psum_yt,
            tc.tile_pool(name="psum_t", bufs=2, space="PSUM") as psum_t,
            tc.tile_pool(name="psum_sn", bufs=1, space="PSUM") as psum_sn,
        ):
            # ---- constants / weights ----
            ident = consts.tile([128, 128], fp32)
            make_identity(nc, ident)
            ident_bf = consts.tile([32, 32], bf16)
            make_identity(nc, ident_bf)
            ones_bf = consts.tile([128, 1], bf16)
            nc.gpsimd.memset(ones_bf[:], 1.0)
            ones_f = consts.tile([128, 1], fp32)
            nc.gpsimd.memset(ones_f[:], 1.0)

            xt_sb = consts.tile([128, KT, B], bf16)
            nc.sync.dma_start(out=xt_sb[:], in_=xt_d[:])
            cs_sb = consts.tile([B, 4, 128], fp32)
            nc.sync.dma_start(out=cs_sb[:], in_=cs_d[:])
            wc_sb = consts.tile([128, KT, F3], bf16)
            for n in range(KT):
                nc.sync.dma_start(out=wc_sb[:, n], in_=wc_d[:, n])
            wp_sb = consts.tile([128, HEADS_PER_CORE, C], bf16)
            for h in range(HEADS_PER_CORE):
                nc.sync.dma_start(out=wp_sb[:, h], in_=wp_d[:, h])

            # ---- qkv = x @ Wc.T  -> psum [16, 768] ----
            qkv_ps = psum_big.tile([B, F3], fp32, tag="big")
            for n in range(KT):
                for c0 in range(0, F3, 512):
                    cw = min(512, F3 - c0)
                    nc.tensor.matmul(
                        out=qkv_ps[:, c0:c0 + cw],
                        lhsT=xt_sb[:, n],
                        rhs=wc_sb[:, n, c0:c0 + cw],
                        start=(n == 0), stop=(n == KT - 1),
                    )
            qkv_sb = sb.tile([B, F3], fp32)
            nc.vector.tensor_copy(qkv_sb[:], qkv_ps[:])

            # ---- RoPE on q and k (f32) ----
            # layout of qkv_sb: [b, 3*2*128]; q cols 0:256, k 256:512, v 512:768
            # within a head: col 2j (even) / 2j+1 (odd), j = 0..63
            cq, sq = cs_sb[:, 0], cs_sb[:, 1]   # [16,128], q copy has SCALE folded
            ck, sk = cs_sb[:, 2], cs_sb[:, 3]
            qrot = sb.tile([B, 256], fp32)
            krot = sb.tile([B, 256], fp32)
            tmp0 = sb.tile([B, 128], fp32)
            tmp1 = sb.tile([B, 128], fp32)
            for (src0, dst, cv, sv) in ((0, qrot, cq, sq), (256, krot, ck, sk)):
                ev = qkv_sb[:, src0:src0 + 256].rearrange("b (j t) -> b t j", t=2)
                dv = dst[:].rearrange("b (j t) -> b t j", t=2)
                e, o = ev[:, 0], ev[:, 1]
                nc.vector.tensor_mul(tmp0[:], e, cv)
                nc.vector.tensor_mul(tmp1[:], o, sv)
                nc.vector.tensor_sub(dv[:, 0], tmp0[:], tmp1[:])
                nc.vector.tensor_mul(tmp0[:], o, cv)
                nc.vector.tensor_mul(tmp1[:], e, sv)
                nc.vector.tensor_add(dv[:, 1], tmp0[:], tmp1[:])

            # new k/v rows out (k after rope, v straight from projection).
            # krot carries SCALE on nothing (unscaled ck/sk) -> true k_new.
            nc.sync.dma_start(out=knew_d[:], in_=krot[:])
            nc.sync.dma_start(out=vnew_d[:], in_=qkv_sb[:, 512:768])

            # ---- qT/kT [128, 32] (d on partitions, pair = h*16+b on free) ----
            qT = sb.tile([128, HB], bf16)
            qTf = sb.tile([128, HB], fp32)
            kTf = sb.tile([128, HB], fp32)
            for h in range(HEADS_PER_CORE):
                cols = slice(h * B, (h + 1) * B)
                tq = psum_t.tile([128, B], fp32, tag="tp")
                nc.tensor.transpose(tq[:], qrot[:, h * 128:(h + 1) * 128],
                                    ident[0:B, 0:B])
                nc.vector.tensor_copy(qT[:, cols], tq[:])
                nc.vector.tensor_copy(qTf[:, cols], tq[:])
                tk = psum_t.tile([128, B], fp32, tag="tp")
                nc.tensor.transpose(tk[:], krot[:, h * 128:(h + 1) * 128],
                                    ident[0:B, 0:B])
                nc.vector.tensor_copy(kTf[:, cols], tk[:])

            # vP [32, 128] pair-major copy of v_new (for the new-token term)
            vP = sb.tile([PAIRS, 128], fp32)
            for h in range(HEADS_PER_CORE):
                # SBUF->SBUF DMA: engine writes must be 32-partition aligned,
                # DMA can target partition offset 16
                nc.sync.dma_start(out=vP[h * B:(h + 1) * B, :],
                                  in_=qkv_sb[:, 512 + h * 128:512 + (h + 1) * 128])

            # ---- new-token scores snew[1, p] = sum_d qT*kT ----
            prodT = sb.tile([128, HB], fp32)
            nc.vector.tensor_mul(prodT[:], qTf[:], kTf[:])
            snew_ps = psum_sn.tile([1, HB], fp32, tag="sn")
            nc.tensor.matmul(out=snew_ps[:], lhsT=ones_f[:], rhs=prodT[:],
                             start=True, stop=True)
            snew_sb = sb.tile([1, HB], fp32)
            nc.vector.tensor_copy(snew_sb[:], snew_ps[:])
            enewT = sb.tile([1, HB], fp32)
            nc.scalar.activation(out=enewT[:], in_=snew_sb[:], func=Exp)
            # pair-major variant for diag(e_new)
            snp_ps = psum_sn.tile([PAIRS, 1], fp32, tag="sn")
            nc.tensor.transpose(snp_ps[:], snew_sb[:], ident[0:1, 0:1])
            enewP = sb.tile([PAIRS, 1], fp32)
            nc.scalar.activation(out=enewP[:], in_=snp_ps[:], func=Exp)
            diag_e = sb.tile([PAIRS, PAIRS], fp32)
            nc.vector.tensor_scalar_mul(diag_e[:], ident[0:PAIRS, 0:PAIRS],
                                        enewP[:])

            # ---- scores over the cache: scoresT [128, NT, 32] psum ----
            scoresT = psum_big.tile([128, NT, HB], fp32, tag="big")
            for b in range(B):
                ktile = kpool.tile([128, HEADS_PER_CORE, S], bf16)
                nc.sync.dma_start(out=ktile[:],
                                  in_=kt_d[b].rearrange("h p s -> p h s"))
                for h in range(HEADS_PER_CORE):
                    p = h * B + b
                    for t in range(NT):
                        nc.tensor.matmul(
                            out=scoresT[:, t, p:p + 1],
                            lhsT=ktile[:, h, t * 128:(t + 1) * 128],
                            rhs=qT[:, p:p + 1],
                            start=True, stop=True,
                        )

            # ---- exp (no max-sub: |scores| <= |q||k|/sqrt(D) ~ 10, f32-safe)
            eT = sb.tile([128, NT, HB], bf16)
            nc.scalar.activation(out=eT[:], in_=scoresT[:], func=Exp)

            # ---- denom[1, p] = ones . eT (+ e_new) ----
            den_ps = psum_sn.tile([1, HB], fp32, tag="sn")
            for t in range(NT):
                nc.tensor.matmul(out=den_ps[:], lhsT=ones_bf[:], rhs=eT[:, t],
                                 start=(t == 0), stop=(t == NT - 1))
            den_sb = sb.tile([1, HB], fp32)
            nc.vector.tensor_add(den_sb[:], den_ps[:], enewT[:])
            dp_ps = psum_sn.tile([PAIRS, 1], fp32, tag="sn")
            nc.tensor.transpose(dp_ps[:], den_sb[:], ident[0:1, 0:1])
            rinvP = sb.tile([PAIRS, 1], fp32)
            nc.vector.reciprocal(rinvP[:], dp_ps[:])

            # ---- yT[d, p] = sum attn V : psum [128, 32] ----
            yT_ps = psum_yt.tile([128, HB], fp32, tag="yt")
            # new-token term first (start=True clears the bank region)
            nc.tensor.matmul(out=yT_ps[:], lhsT=vP[:], rhs=diag_e[:],
                             start=True, stop=False)
            for b in range(B):
                vtile = vpool.tile([128, HEADS_PER_CORE, NT, 128], bf16)
                nc.sync.dma_start(out=vtile[:],
                                  in_=vt_d[b].rearrange("h p n d -> p h n d"))
                for h in range(HEADS_PER_CORE):
                    p = h * B + b
                    for t in range(NT):
                        nc.tensor.matmul(
                            out=yT_ps[:, p:p + 1],
                            lhsT=vtile[:, h, t],
                            rhs=eT[:, t, p:p + 1],
                            start=False, stop=(t == NT - 1),
                        )

            # ---- normalize: y_pair = yT.T * rinv; back to yTn [128, 32] bf16
            yT_sb = sb.tile([128, HB], fp32)
            nc.vector.tensor_copy(yT_sb[:], yT_ps[:])
            ypair_ps = psum_yt.tile([PAIRS, 128], fp32, tag="yt")
            nc.tensor.transpose(ypair_ps[:], yT_sb[:], ident[:, :])
            ypair_sb = sb.tile([PAIRS, 128], fp32)
            nc.vector.tensor_scalar_mul(ypair_sb[:], ypair_ps[:], rinvP[:])
            ytn_ps = psum_yt.tile([128, HB], fp32, tag="yt")
            nc.tensor.transpose(ytn_ps[:], ypair_sb[:], ident[0:PAIRS, 0:PAIRS])
            ytn_sb = sb.tile([128, HB], bf16)
            nc.vector.tensor_copy(ytn_sb[:], ytn_ps[:])

            # ---- y_partial = y_heads @ Wp_slice.T : [16, 2048] ----
            out_ps = psum_big.tile([B, C], fp32, tag="big")
            for h in range(HEADS_PER_CORE):
                for c0 in range(0, C, 512):
                    nc.tensor.matmul(
                        out=out_ps[:, c0:c0 + 512],
                        lhsT=ytn_sb[:, h * B:(h + 1) * B],
                        rhs=wp_sb[:, h, c0:c0 + 512],
                        start=(h == 0), stop=(h == HEADS_PER_CORE - 1),
                    )
            out_sb = sb.tile([B, C], fp32)
            nc.vector.tensor_copy(out_sb[:], out_ps[:])
            nc.sync.dma_start(out=yp_d[:], in_=out_sb[:])

    nc.compile()
    return nc


def _get_program(S):
    if S not in _cache:
        _cache[S] = _build_program(S)
    return _cache[S]


def _prep_core_inputs(x, W_attn, W_proj, rope, cache_k, cache_v, S):
    """Build per-core input dicts (host-side shard + transpose + bf16 cast)."""
    x2 = np.asarray(x, np.float32).reshape(B, C)
    xt = np.ascontiguousarray(x2.reshape(B, C // 128, 128).transpose(2, 1, 0)).astype(BF16)

    c64 = np.asarray(rope, np.float32)[0, :, 0]
    s64 = np.asarray(rope, np.float32)[0, :, 1]
    c128 = np.tile(c64, 2)  # [128] — cos per (head-local j), heads concatenated
    s128 = np.tile(s64, 2)
    cs = np.empty((B, 4, 128), np.float32)
    cs[:, 0] = c128 * SCALE
    cs[:, 1] = s128 * SCALE
    cs[:, 2] = c128
    cs[:, 3] = s128

    W_attn = np.asarray(W_attn, np.float32)
    W_proj = np.asarray(W_proj, np.float32)
    NT = S // 128

    in_maps = []
    for core in range(N_CORES):
        h0 = core * HEADS_PER_CORE
        r0 = h0 * D
        r1 = (h0 + HEADS_PER_CORE) * D
        wc_sel = np.concatenate(
            [W_attn[r0:r1], W_attn[C + r0:C + r1], W_attn[2 * C + r0:2 * C + r1]], axis=0
        )  # [768, 2048]
        wc = np.ascontiguousarray(
            wc_sel.reshape(3 * HEADS_PER_CORE * D, C // 128, 128).transpose(2, 1, 0)
        ).astype(BF16)  # [128, 16, 768]
        wp = np.ascontiguousarray(
            W_proj[:, r0:r1].T.reshape(HEADS_PER_CORE, 128, C).transpose(1, 0, 2)
        ).astype(BF16)  # [128, 2, 2048]
        kt = np.ascontiguousarray(
            cache_k[:, h0:h0 + HEADS_PER_CORE, :S, :].transpose(0, 1, 3, 2)
        ).astype(BF16)  # [16, 2, 128, S]
        vt = np.ascontiguousarray(
            np.asarray(cache_v[:, h0:h0 + HEADS_PER_CORE, :S, :])
            .reshape(B, HEADS_PER_CORE, NT, 128, D).transpose(0, 1, 3, 2, 4)
        ).astype(BF16)  # [16, 2, 128, NT, 128]
        in_maps.append(dict(xt=xt, wc=wc, wp=wp, kt=kt, vt=vt, cs=cs))
    return in_maps


def _numpy_fallback(x, W_attn, W_proj, rope, cache_k, cache_v, mask, input_pos):
    """Exact (slow) host implementation, mirrors reference.py."""
    x = np.asarray(x, np.float32)
    B_, T_, C_ = x.shape
    H_ = NH
    D_ = C_ // H_
    qkv = x @ np.asarray(W_attn, np.float32).T
    q, k, v = np.split(qkv, 3, axis=2)
    q = q.reshape(B_, T_, H_, D_)
    k = k.reshape(B_, T_, H_, D_)
    v = v.reshape(B_, T_, H_, D_)

    def rope_apply(t):
        ts_ = t.reshape(t.shape[:-1] + (D_ // 2, 2))
        rc = np.asarray(rope, np.float32)[None, :, None, :, :]
        out = np.stack(
            [ts_[..., 0] * rc[..., 0] - ts_[..., 1] * rc[..., 1],
             ts_[..., 1] * rc[..., 0] + ts_[..., 0] * rc[..., 1]], axis=-1)
        return out.reshape(t.shape)

    q = rope_apply(q).transpose(0, 2, 1, 3)
    k = rope_apply(k).transpose(0, 2, 1, 3)
    v = v.transpose(0, 2, 1, 3)
    pos = np.asarray(input_pos).reshape(-1)
    k_full = np.array(cache_k, np.float32, copy=True)
    v_full = np.array(cache_v, np.float32, copy=True)
    k_full[:, :, pos, :] = k
    v_full[:, :, pos, :] = v
    scores = np.einsum('bhqd,bhkd->bhqk', q, k_full) / math.sqrt(D_)
    m = np.asarray(mask).reshape(-1)[None, None, None, :]
    scores = np.where(m, scores, np.finfo(np.float32).min)
    scores = scores - scores.max(axis=-1, keepdims=True)
    e = np.exp(scores)
    attn = e / e.sum(axis=-1, keepdims=True)
    y = np.einsum('bhqk,bhkd->bhqd', attn, v_full)
    y = y.transpose(0, 2, 1, 3).reshape(B_, T_, C_)
    y = y @ np.asarray(W_proj, np.float32).T
    return (y.astype(np.float32), k_full, v_full)


def kernel(x, W_attn, W_proj, rope, cache_k, cache_v, mask, input_pos,
           _trace=False, _trace_kwargs=None):
    from concourse.bass_utils import run_bass_kernel_spmd

    x = np.asarray(x)
    mask_row = np.asarray(mask).reshape(-1)
    pos = int(np.asarray(input_pos).reshape(-1)[0])
    S_eff = int(mask_row.sum())

    ok = (
        x.shape == (B, 1, C)
        and np.asarray(cache_k).shape == (B, NH, MAX_SEQ, D)
        and mask_row.shape[0] == MAX_SEQ
        and bool(mask_row[:S_eff].all())        # prefix mask
        and pos == S_eff - 1                     # decode step writes last pos
        and pos % 512 == 0 and pos > 0
    )
    if not ok:
        return _numpy_fallback(x, W_attn, W_proj, rope, cache_k, cache_v,
                               mask, input_pos)

    S = pos  # cache rows attended with their original contents
    nc = _get_program(S)
    in_maps = _prep_core_inputs(x, W_attn, W_proj, rope, cache_k, cache_v, S)
    res = run_bass_kernel_spmd(nc, in_maps, list(range(N_CORES)),
                               trace=_trace, **(_trace_kwargs or {}))

    y = np.zeros((B, C), np.float64)
    for core in range(N_CORES):
        y += res.results[core]["yp"].astype(np.float64)
    y = y.astype(np.float32).reshape(B, 1, C)

    k_full = np.array(cache_k, np.float32, copy=True)
    v_full = np.array(cache_v, np.float32, copy=True)
    for core in range(N_CORES):
        h0 = core * HEADS_PER_CORE
        k_full[:, h0:h0 + HEADS_PER_CORE, pos, :] = \
            res.results[core]["knew"].reshape(B, HEADS_PER_CORE, D)
        v_full[:, h0:h0 + HEADS_PER_CORE, pos, :] = \
            res.results[core]["vnew"].reshape(B, HEADS_PER_CORE, D)

    if _trace:
        kernel._last_results = res
    return (y, k_full, v_full)
